# revision 16
# baseline (speedup 1.0000x reference)
"""GPT-2 AR decode kernel for Trainium2 (Bass/Tile).

Strategy: single-NeuronCore full-recompute forward pass per decode step
(static [144] token buffer), host-orchestrated sampling loop with exact
jax PRNG semantics. The transformer forward (all matmuls, LN, attention,
gelu, LM head) runs on device; the host does embedding gather, top-k
threshold, and categorical sampling between launches.

Layout: activations are d-major: 6 SBUF tiles [128, 144] (partition = d
chunk, free = token). Weights [din, dout] load directly as matmul lhsT.
"""

import numpy as np

L, D, H, HD, FF, V = 6, 768, 12, 64, 3072, 2051
NUM_SEM, NUM_AC = 1024, 1024
TEMP = 1.0
P = 128          # prompt length
S = 16           # decode steps
T = P + S        # 144 static token buffer
DC = D // 128    # 6 d-chunks
VPAD = 17 * 128  # 2176 padded vocab
FT = FF // 128   # 24 ffn tiles
TW = 144         # token tile width
EPS = 1e-5
NEG = -1.0e30

_CACHE = {}


def _build_nc():
    import concourse.bass as bass
    import concourse.bacc as bacc
    import concourse.mybir as mybir
    from concourse import tile
    from concourse.alu_op_type import AluOpType as op

    dt = mybir.dt.float32
    AF = mybir.ActivationFunctionType
    AX = mybir.AxisListType.X

    nc = bacc.Bacc(None, target_bir_lowering=False)


    # ---- DRAM parameters ----
    x6_d = nc.declare_dram_parameter("x6", [DC, 128, TW], dt, isOutput=False)
    aw_d = nc.declare_dram_parameter("attn_w", [L, D, 3 * D], dt, isOutput=False)
    pw_d = nc.declare_dram_parameter("proj_w", [L, D, D], dt, isOutput=False)
    fw_d = nc.declare_dram_parameter("fc_w", [L, D, FF], dt, isOutput=False)
    gw_d = nc.declare_dram_parameter("fcp_w", [L, FF, D], dt, isOutput=False)
    wt_d = nc.declare_dram_parameter("wteT", [D, VPAD], dt, isOutput=False)
    # per-layer packed scale/bias columns: ln1g ln1b ln2g ln2b (6 cols each)
    ln_d = nc.declare_dram_parameter("ln_all", [L, 128, 24], dt, isOutput=False)
    lnf_d = nc.declare_dram_parameter("lnf_all", [128, 12], dt, isOutput=False)
    # biases: qk (12 cols), proj (6), fc (24), fcp (6)
    bs_d = nc.declare_dram_parameter("bias_all", [L, 128, 48], dt, isOutput=False)
    ones_d = nc.declare_dram_parameter("ones", [128, 128], dt, isOutput=False)
    iden_d = nc.declare_dram_parameter("iden", [128, 128], dt, isOutput=False)
    # causal additive masks for the two q-tiles
    mk_d = nc.declare_dram_parameter("cmask", [2, 128, TW], dt, isOutput=False)
    out_d = nc.declare_dram_parameter("logits", [17, 128, TW], dt, isOutput=True)

    with tile.TileContext(nc) as tc:
        with (
            tc.tile_pool(name="pers", bufs=1) as pp,
            tc.tile_pool(name="awp", bufs=6) as awp,
            tc.tile_pool(name="fwp", bufs=8) as fwp,
            tc.tile_pool(name="w768", bufs=7) as w768p,
            tc.tile_pool(name="qkp", bufs=12) as qkp,
            tc.tile_pool(name="gep", bufs=24) as gep,
            tc.tile_pool(name="trn", bufs=8) as trp,
            tc.tile_pool(name="aop", bufs=1) as aop,
            tc.tile_pool(name="vtp", bufs=24) as vtp,
            tc.tile_pool(name="psacc", bufs=6, space="PSUM") as psaccp,
            tc.tile_pool(name="pswk", bufs=2, space="PSUM") as pswkp,
        ):
            def psum(p_=128, f=TW, tag=None):
                return pswkp.tile([p_, f], dt, tag="wk", name="wk")

            def psacc(tag=None):
                return psaccp.tile([128, TW], dt, tag="acc", name="acc")

            h = [pp.tile([128, TW], dt, tag=f"h{c}", name=f"h{c}") for c in range(DC)]
            xn = [pp.tile([128, TW], dt, tag=f"xn{c}", name=f"xn{c}") for c in range(DC)]
            ones = pp.tile([128, 128], dt, tag="ones", name="ones")
            iden = pp.tile([128, 128], dt, tag="iden", name="iden")
            cmask = [pp.tile([128, TW], dt, tag=f"cm{q}", name=f"cm{q}") for q in range(2)]
            lnb = pp.tile([128, 24], dt, tag="lnb", name="lnb")
            bsb = pp.tile([128, 48], dt, tag="bsb", name="bsb")
            rowbuf = pp.tile([1, 6 * TW], dt, tag="rowbuf", name="rowbuf")
            mrb = [pp.tile([128, TW], dt, tag=f"mrb{i}", name=f"mrb{i}") for i in range(2)]

            nc.sync.dma_start(ones[:], ones_d[:, :])
            nc.sync.dma_start(iden[:], iden_d[:, :])
            for q in range(2):
                nc.sync.dma_start(cmask[q][:], mk_d[q, :, :])
            for c in range(DC):
                nc.sync.dma_start(h[c][:], x6_d[c, :, :])

            def layernorm(src, dst, g_ap, b_ap):
                """dst[c] = (src[c]-mean)/sqrt(var+eps) * g + b  (over d)."""
                r1 = psum(1, TW)
                r2 = psum(1, TW)
                sq = [trp.tile([128, TW], dt, tag="trn", name="trn") for _ in range(DC)]
                for c in range(DC):
                    nc.scalar.activation(sq[c][:], src[c][:], AF.Square)
                for c in range(DC):
                    nc.tensor.matmul(r1[:], (ones[:, 0:1]), (src[c][:]),
                                     start=(c == 0), stop=(c == DC - 1))
                for c in range(DC):
                    nc.tensor.matmul(r2[:], (ones[:, 0:1]), (sq[c][:]),
                                     start=(c == 0), stop=(c == DC - 1))
                # rows: mean, var, 1/sqrt(var+eps) with one Newton polish
                mrow = rowbuf[0:1, 0:TW]
                vrow = rowbuf[0:1, TW:2 * TW]
                srow = rowbuf[0:1, 2 * TW:3 * TW]
                rrow = rowbuf[0:1, 3 * TW:4 * TW]
                t1 = rowbuf[0:1, 4 * TW:5 * TW]
                nc.vector.tensor_scalar(mrow, r1[:], 1.0 / D, None, op.mult)
                nc.vector.tensor_scalar(vrow, r2[:], 1.0 / D, None, op.mult)
                nc.vector.tensor_tensor(t1, mrow, mrow, op.mult)
                nc.vector.tensor_tensor(vrow, vrow, t1, op.subtract)
                # srow = var + eps ; rrow ~= rsqrt(srow) ; newton polish
                nc.vector.tensor_scalar(srow, vrow, EPS, None, op.add)
                nc.scalar.activation(rrow, srow, AF.Sqrt)
                nc.vector.reciprocal(t1, rrow)
                # newton: r = r*(1.5 - 0.5*s*r*r)
                nc.vector.tensor_tensor(rrow, t1, t1, op.mult)
                nc.vector.tensor_tensor(rrow, rrow, srow, op.mult)
                nc.vector.tensor_scalar(rrow, rrow, -0.5, 1.5, op.mult, op.add)
                nc.vector.tensor_tensor(rrow, rrow, t1, op.mult)
                # broadcast mean, rstd to [128, T] (K=1 matmul), move to SBUF
                bm = psum(128, TW)
                br = psum(128, TW)
                nc.tensor.matmul(bm[:], (ones[0:1, :]), (mrow), start=True, stop=True)
                nc.tensor.matmul(br[:], (ones[0:1, :]), (rrow), start=True, stop=True)
                nc.scalar.activation(mrb[0][:], bm[:], AF.Copy)
                nc.scalar.activation(mrb[1][:], br[:], AF.Copy)
                for c in range(DC):
                    tt = trp.tile([128, TW], dt, tag="trn", name="trn")
                    nc.gpsimd.tensor_tensor(tt[:], src[c][:], mrb[0][:], op.subtract)
                    nc.gpsimd.tensor_tensor(tt[:], tt[:], mrb[1][:], op.mult)
                    nc.vector.tensor_scalar(dst[c][:], tt[:], g_ap(c), b_ap(c),
                                            op.mult, op.add)

            for l in range(L):
                nc.sync.dma_start(lnb[:], ln_d[l, :, :])
                nc.sync.dma_start(bsb[:], bs_d[l, :, :])

                layernorm(h, xn,
                          lambda c: lnb[:, c:c + 1],
                          lambda c: lnb[:, 6 + c:6 + c + 1])

                # ---- attention ----
                awt = [awp.tile([128, 3 * D], dt, tag="aw", name="aw") for c in range(DC)]
                for c in range(DC):
                    nc.sync.dma_start(awt[c][:], aw_d[l, c * 128:(c + 1) * 128, :])

                # q,k projections: 12 dout tiles [128, T]
                qk = [qkp.tile([128, TW], dt, tag="qk", name="qk") for _ in range(12)]
                for dtile in range(12):
                    ps = psum()
                    for c in range(DC):
                        nc.tensor.matmul(ps[:], (awt[c][:, dtile * 128:(dtile + 1) * 128]),
                                         (xn[c][:]), start=(c == 0), stop=(c == DC - 1))
                    nc.scalar.activation(qk[dtile][:], ps[:], AF.Identity,
                                         bias=bsb[:, dtile:dtile + 1])
                # vT: per head, two token tiles [128,64],[16,64]
                vt = []
                for hh in range(H):
                    row = []
                    for q in range(2):
                        tw = 128 if q == 0 else TW - 128
                        ps = psum(128, 64)
                        for c in range(DC):
                            nc.tensor.matmul(
                                ps[0:tw, :],
                                (xn[c][:, q * 128:q * 128 + tw]),
                                (awt[c][:, 2 * D + hh * HD:2 * D + (hh + 1) * HD]),
                                start=(c == 0), stop=(c == DC - 1))
                        sb = vtp.tile([128, 64], dt, tag="vt", name="vt")
                        nc.scalar.activation(sb[0:tw, :], ps[0:tw, :], AF.Copy)
                        row.append(sb)
                    vt.append(row)

                # scores + softmax + AV per head
                aout = [aop.tile([128, TW], dt, tag=f"ao{i}", name=f"ao{i}") for i in range(DC)]
                for hh in range(H):
                    dtile, ro = hh // 2, (hh % 2) * 64
                    q_ap = qk[dtile][ro:ro + 64, :]
                    k_ap = qk[6 + dtile][ro:ro + 64, :]
                    pT = [trp.tile([128, TW], dt, tag="trn", name="trn") for _ in range(2)]
                    for q in range(2):
                        tw = 128 if q == 0 else TW - 128
                        ps = psum()
                        nc.tensor.matmul(ps[0:tw, :], (q_ap[:, q * 128:q * 128 + tw]),
                                         (k_ap), start=True, stop=True)
                        ss = trp.tile([128, TW], dt, tag="trn", name="trn")
                        nc.vector.tensor_tensor(ss[0:tw, :], ps[0:tw, :],
                                                cmask[q][0:tw, :], op.add)
                        mx = trp.tile([128, 2], dt, tag="mx", name="mx")
                        nc.vector.reduce_max(mx[0:tw, 0:1], ss[0:tw, :], AX)
                        nc.vector.tensor_scalar(mx[0:tw, 1:2], mx[0:tw, 0:1],
                                                -0.125, None, op.mult)
                        pe_ = trp.tile([128, TW], dt, tag="trn", name="trn")
                        nc.scalar.activation(pe_[0:tw, :], ss[0:tw, :], AF.Exp,
                                             bias=mx[0:tw, 1:2], scale=0.125)
                        nc.vector.reduce_sum(mx[0:tw, 0:1], pe_[0:tw, :], AX)
                        nc.vector.reciprocal(mx[0:tw, 0:1], mx[0:tw, 0:1])
                        nc.vector.tensor_scalar(pe_[0:tw, :], pe_[0:tw, :],
                                                mx[0:tw, 0:1], None, op.mult)
                        # transpose pe_[0:tw, 0:T] into pT tiles
                        for kb in range(2):
                            kw = 128 if kb == 0 else TW - 128
                            pst = psum(128, 128)
                            nc.tensor.transpose(pst[0:kw, 0:tw],
                                                pe_[0:tw, kb * 128:kb * 128 + kw],
                                                iden[0:tw, 0:tw])
                            nc.vector.tensor_copy(
                                pT[kb][0:kw, q * 128:q * 128 + tw],
                                pst[0:kw, 0:tw])
                    pa = psum(128, TW)
                    for kb in range(2):
                        kw = 128 if kb == 0 else TW - 128
                        nc.tensor.matmul(pa[0:64, :], (vt[hh][kb][0:kw, :]),
                                         (pT[kb][0:kw, :]), start=(kb == 0),
                                         stop=(kb == 1))
                    nc.scalar.activation(aout[dtile][ro:ro + 64, :], pa[0:64, :],
                                         AF.Copy)

                # proj + residual
                pwt = [w768p.tile([128, D], dt, tag="w768", name="w768") for _ in range(DC)]
                for c in range(DC):
                    nc.sync.dma_start(pwt[c][:], pw_d[l, c * 128:(c + 1) * 128, :])
                for co in range(DC):
                    ps = psum()
                    for c in range(DC):
                        nc.tensor.matmul(ps[:], (pwt[c][:, co * 128:(co + 1) * 128]),
                                         (aout[c][:]), start=(c == 0), stop=(c == DC - 1))
                    nc.vector.scalar_tensor_tensor(h[co][:], ps[:],
                                                   bsb[:, 12 + co:12 + co + 1],
                                                   h[co][:], op.add, op.add)

                layernorm(h, xn,
                          lambda c: lnb[:, 12 + c:12 + c + 1],
                          lambda c: lnb[:, 18 + c:18 + c + 1])

                # fc + gelu
                ge = [gep.tile([128, TW], dt, tag="ge", name="ge") for _ in range(FT)]
                for half in range(2):
                    fwt = [fwp.tile([128, FF // 2], dt, tag="fw", name="fw") for _ in range(DC)]
                    for c in range(DC):
                        nc.sync.dma_start(
                            fwt[c][:],
                            fw_d[l, c * 128:(c + 1) * 128,
                                 half * (FF // 2):(half + 1) * (FF // 2)])
                    for ft in range(half * 12, half * 12 + 12):
                        fo = ft * 128 - half * (FF // 2)
                        ps = psum()
                        for c in range(DC):
                            nc.tensor.matmul(ps[:], (fwt[c][:, fo:fo + 128]),
                                             (xn[c][:]), start=(c == 0),
                                             stop=(c == DC - 1))
                        nc.scalar.activation(ge[ft][:], ps[:], AF.Gelu_apprx_tanh,
                                             bias=bsb[:, 18 + ft:18 + ft + 1])
                # fcp + residual, staged in 4 groups of 6 k-chunks
                psf = [psacc() for co in range(DC)]
                for gg in range(4):
                    gwt = [w768p.tile([128, D], dt, tag="w768", name="w768") for _ in range(6)]
                    for i in range(6):
                        kk = gg * 6 + i
                        nc.sync.dma_start(gwt[i][:],
                                          gw_d[l, kk * 128:(kk + 1) * 128, :])
                    for co in range(DC):
                        for i in range(6):
                            kk = gg * 6 + i
                            nc.tensor.matmul(
                                psf[co][:], (gwt[i][:, co * 128:(co + 1) * 128]),
                                (ge[kk][:]), start=(gg == 0 and i == 0),
                                stop=(gg == 3 and i == 5))
                for co in range(DC):
                    nc.vector.scalar_tensor_tensor(h[co][:], psf[co][:],
                                                   bsb[:, 42 + co:42 + co + 1],
                                                   h[co][:], op.add, op.add)

            # final LN + LM head
            nc.sync.dma_start(lnb[:, 0:12], lnf_d[:, :])
            layernorm(h, xn,
                      lambda c: lnb[:, c:c + 1],
                      lambda c: lnb[:, 6 + c:6 + c + 1])
            for vtile in range(17):
                wvt = w768p.tile([128, D], dt, tag="w768", name="w768")
                # wteT slice [768, 128] -> need [128(dchunk) x ...]: DMA per chunk
                lg = gep.tile([128, TW], dt, tag="ge", name="ge")
                ps = psum()
                for c in range(DC):
                    nc.sync.dma_start(
                        wvt[:, c * 128:(c + 1) * 128],
                        wt_d[c * 128:(c + 1) * 128, vtile * 128:(vtile + 1) * 128])
                for c in range(DC):
                    nc.tensor.matmul(ps[:], (wvt[:, c * 128:(c + 1) * 128]),
                                     (xn[c][:]), start=(c == 0), stop=(c == DC - 1))
                nc.scalar.activation(lg[:], ps[:], AF.Copy)
                nc.sync.dma_start(out_d[vtile, :, :], lg[:])

    nc.compile()
    return nc


def _prep_static(wte, wpe, ln1_g, ln1_b, attn_w, proj_w, fc_w, fcp_w,
                 ln2_g, ln2_b, lnf_g, lnf_b, attn_b, proj_b, fc_b, fcp_b):
    f32 = np.float32
    ln_all = np.zeros((L, 128, 24), f32)
    bias_all = np.zeros((L, 128, 48), f32)
    for l in range(L):
        ln_all[l, :, 0:6] = ln1_g[l].reshape(6, 128).T
        ln_all[l, :, 6:12] = ln1_b[l].reshape(6, 128).T
        ln_all[l, :, 12:18] = ln2_g[l].reshape(6, 128).T
        ln_all[l, :, 18:24] = ln2_b[l].reshape(6, 128).T
        bias_all[l, :, 0:12] = attn_b[l, :2 * D].reshape(12, 128).T
        bias_all[l, :, 12:18] = proj_b[l].reshape(6, 128).T
        bias_all[l, :, 18:42] = fc_b[l].reshape(24, 128).T
        bias_all[l, :, 42:48] = fcp_b[l].reshape(6, 128).T
    lnf_all = np.zeros((128, 12), f32)
    lnf_all[:, 0:6] = lnf_g.reshape(6, 128).T
    lnf_all[:, 6:12] = lnf_b.reshape(6, 128).T
    wteT = np.zeros((D, VPAD), f32)
    wteT[:, :V] = wte.T
    cmask = np.zeros((2, 128, TW), f32)
    for q in range(2):
        for p_ in range(128):
            qpos = q * 128 + p_
            cmask[q, p_, qpos + 1:] = NEG
    ones = np.ones((128, 128), f32)
    iden = np.eye(128, dtype=f32)
    return dict(attn_w=np.ascontiguousarray(attn_w, f32),
                proj_w=np.ascontiguousarray(proj_w, f32),
                fc_w=np.ascontiguousarray(fc_w, f32),
                fcp_w=np.ascontiguousarray(fcp_w, f32),
                wteT=wteT, ln_all=ln_all, lnf_all=lnf_all,
                bias_all=bias_all, ones=ones, iden=iden, cmask=cmask)


def kernel(wte, wpe, ln1_g, ln1_b, attn_w, attn_b, proj_w, proj_b,
           ln2_g, ln2_b, fc_w, fc_b, fcp_w, fcp_b, lnf_g, lnf_b,
           first_idx, pos_ids, max_len, topk, _collect=None):
    import jax
    import jax.numpy as jnp
    from concourse.bass_utils import run_bass_kernel_spmd

    wte = np.asarray(wte, np.float32)
    wpe = np.asarray(wpe, np.float32)
    first_idx = np.asarray(first_idx, np.int32)
    max_len = int(max_len)
    kk = min(int(topk), V)
    assert max_len == S and first_idx.shape == (1, P)

    if "nc" not in _CACHE:
        _CACHE["nc"] = _build_nc()
    nc = _CACHE["nc"]

    static = _prep_static(np.asarray(wte), wpe,
                          np.asarray(ln1_g, np.float32), np.asarray(ln1_b, np.float32),
                          np.asarray(attn_w, np.float32), np.asarray(proj_w, np.float32),
                          np.asarray(fc_w, np.float32), np.asarray(fcp_w, np.float32),
                          np.asarray(ln2_g, np.float32), np.asarray(ln2_b, np.float32),
                          np.asarray(lnf_g, np.float32), np.asarray(lnf_b, np.float32),
                          np.asarray(attn_b, np.float32), np.asarray(proj_b, np.float32),
                          np.asarray(fc_b, np.float32), np.asarray(fcp_b, np.float32))

    pos_full = np.arange(T, dtype=np.int32)
    buf = np.zeros(T, np.int32)
    buf[:P] = first_idx[0]
    cpu0 = jax.devices("cpu")[0]
    with jax.default_device(cpu0):
        keys = jax.random.split(jax.random.key(1), S)
    band = np.arange(V)
    inf_mask = (band <= NUM_SEM) | (band == NUM_SEM + 1 + NUM_AC)

    cpu = jax.devices("cpu")[0]
    step_logits = np.zeros((S, 1, V), np.float32)

    for s in range(S):
        x = wte[buf] + wpe[pos_full]          # [T, 768]
        xp = np.zeros((TW, D), np.float32)
        xp[:T] = x
        x6 = np.ascontiguousarray(xp.T.reshape(DC, 128, TW), np.float32)
        in_map = dict(static, x6=x6)
        import os as _os
        _tr = bool(_os.environ.get("BASS_KERNEL_TRACE")) and s == 0
        res = run_bass_kernel_spmd(nc, [in_map], core_ids=[0], trace=_tr)
        lg = res.results[0]["logits"]          # [17, 128, T]
        if _collect is not None:
            _collect.append(res)
        cur = P + s
        row = lg[:, :, cur - 1].reshape(VPAD)[:V].astype(np.float32)
        row = row * TEMP
        row = np.where(inf_mask, -np.inf, row)
        thresh = np.partition(row, V - kk)[V - kk]
        row = np.where(row < thresh, -np.inf, row).astype(np.float32)
        step_logits[s, 0] = row
        with jax.default_device(cpu):
            samp = int(jax.random.categorical(keys[s], jnp.asarray(row)[None, :])[0])
        buf[cur] = samp

    idx_buf = buf[None, :].astype(np.int32)
    return idx_buf, step_logits


# revision 17
# speedup vs baseline: 1.0015x; 1.0015x over previous
"""GPT-2 AR decode kernel for Trainium2 (Bass/Tile).

Strategy: single-NeuronCore full-recompute forward pass per decode step
(static [144] token buffer), host-orchestrated sampling loop with exact
jax PRNG semantics. The transformer forward (all matmuls, LN, attention,
gelu, LM head) runs on device; the host does embedding gather, top-k
threshold, and categorical sampling between launches.

Layout: activations are d-major: 6 SBUF tiles [128, 144] (partition = d
chunk, free = token). Weights [din, dout] load directly as matmul lhsT.
"""

import numpy as np

L, D, H, HD, FF, V = 6, 768, 12, 64, 3072, 2051
NUM_SEM, NUM_AC = 1024, 1024
TEMP = 1.0
P = 128          # prompt length
S = 16           # decode steps
T = P + S        # 144 static token buffer
DC = D // 128    # 6 d-chunks
VPAD = 17 * 128  # 2176 padded vocab
FT = FF // 128   # 24 ffn tiles
TW = 144         # token tile width
EPS = 1e-5
NEG = -1.0e30

_CACHE = {}


def _build_nc():
    import concourse.bass as bass
    import concourse.bacc as bacc
    import concourse.mybir as mybir
    from concourse import tile
    from concourse.alu_op_type import AluOpType as op

    dt = mybir.dt.float32
    AF = mybir.ActivationFunctionType
    AX = mybir.AxisListType.X

    nc = bacc.Bacc(None, target_bir_lowering=False)


    # ---- DRAM parameters ----
    x6_d = nc.declare_dram_parameter("x6", [DC, 128, TW], dt, isOutput=False)
    aw_d = nc.declare_dram_parameter("attn_w", [L, D, 3 * D], dt, isOutput=False)
    pw_d = nc.declare_dram_parameter("proj_w", [L, D, D], dt, isOutput=False)
    fw_d = nc.declare_dram_parameter("fc_w", [L, D, FF], dt, isOutput=False)
    gw_d = nc.declare_dram_parameter("fcp_w", [L, FF, D], dt, isOutput=False)
    wt_d = nc.declare_dram_parameter("wteT", [D, VPAD], dt, isOutput=False)
    # per-layer packed scale/bias columns: ln1g ln1b ln2g ln2b (6 cols each)
    ln_d = nc.declare_dram_parameter("ln_all", [L, 128, 24], dt, isOutput=False)
    lnf_d = nc.declare_dram_parameter("lnf_all", [128, 12], dt, isOutput=False)
    # biases: qk (12 cols), proj (6), fc (24), fcp (6)
    bs_d = nc.declare_dram_parameter("bias_all", [L, 128, 48], dt, isOutput=False)
    ones_d = nc.declare_dram_parameter("ones", [128, 128], dt, isOutput=False)
    iden_d = nc.declare_dram_parameter("iden", [128, 128], dt, isOutput=False)
    # causal additive masks for the two q-tiles
    mk_d = nc.declare_dram_parameter("cmask", [2, 128, TW], dt, isOutput=False)
    out_d = nc.declare_dram_parameter("logits", [17, 128, 17], dt, isOutput=True)

    with tile.TileContext(nc) as tc:
        with (
            tc.tile_pool(name="pers", bufs=1) as pp,
            tc.tile_pool(name="awp", bufs=6) as awp,
            tc.tile_pool(name="fwp", bufs=8) as fwp,
            tc.tile_pool(name="w768", bufs=7) as w768p,
            tc.tile_pool(name="qkp", bufs=12) as qkp,
            tc.tile_pool(name="gep", bufs=24) as gep,
            tc.tile_pool(name="trn", bufs=8) as trp,
            tc.tile_pool(name="aop", bufs=1) as aop,
            tc.tile_pool(name="vtp", bufs=24) as vtp,
            tc.tile_pool(name="psacc", bufs=6, space="PSUM") as psaccp,
            tc.tile_pool(name="pswk", bufs=2, space="PSUM") as pswkp,
        ):
            def psum(p_=128, f=TW, tag=None):
                return pswkp.tile([p_, f], dt, tag="wk", name="wk")

            def psacc(tag=None):
                return psaccp.tile([128, TW], dt, tag="acc", name="acc")

            h = [pp.tile([128, TW], dt, tag=f"h{c}", name=f"h{c}") for c in range(DC)]
            xn = [pp.tile([128, TW], dt, tag=f"xn{c}", name=f"xn{c}") for c in range(DC)]
            ones = pp.tile([128, 128], dt, tag="ones", name="ones")
            iden = pp.tile([128, 128], dt, tag="iden", name="iden")
            cmask = [pp.tile([128, TW], dt, tag=f"cm{q}", name=f"cm{q}") for q in range(2)]
            lnb = pp.tile([128, 24], dt, tag="lnb", name="lnb")
            bsb = pp.tile([128, 48], dt, tag="bsb", name="bsb")
            rowbuf = pp.tile([1, 6 * TW], dt, tag="rowbuf", name="rowbuf")
            mrb = [pp.tile([128, TW], dt, tag=f"mrb{i}", name=f"mrb{i}") for i in range(2)]

            nc.sync.dma_start(ones[:], ones_d[:, :])
            nc.sync.dma_start(iden[:], iden_d[:, :])
            for q in range(2):
                nc.sync.dma_start(cmask[q][:], mk_d[q, :, :])
            for c in range(DC):
                nc.sync.dma_start(h[c][:], x6_d[c, :, :])

            def layernorm(src, dst, g_ap, b_ap):
                """dst[c] = (src[c]-mean)/sqrt(var+eps) * g + b  (over d)."""
                r1 = psum(1, TW)
                r2 = psum(1, TW)
                sq = [trp.tile([128, TW], dt, tag="trn", name="trn") for _ in range(DC)]
                for c in range(DC):
                    nc.scalar.activation(sq[c][:], src[c][:], AF.Square)
                for c in range(DC):
                    nc.tensor.matmul(r1[:], (ones[:, 0:1]), (src[c][:]),
                                     start=(c == 0), stop=(c == DC - 1))
                for c in range(DC):
                    nc.tensor.matmul(r2[:], (ones[:, 0:1]), (sq[c][:]),
                                     start=(c == 0), stop=(c == DC - 1))
                # rows: mean, var, 1/sqrt(var+eps) with one Newton polish
                mrow = rowbuf[0:1, 0:TW]
                vrow = rowbuf[0:1, TW:2 * TW]
                srow = rowbuf[0:1, 2 * TW:3 * TW]
                rrow = rowbuf[0:1, 3 * TW:4 * TW]
                t1 = rowbuf[0:1, 4 * TW:5 * TW]
                nc.vector.tensor_scalar(mrow, r1[:], 1.0 / D, None, op.mult)
                nc.vector.tensor_scalar(vrow, r2[:], 1.0 / D, None, op.mult)
                nc.vector.tensor_tensor(t1, mrow, mrow, op.mult)
                nc.vector.tensor_tensor(vrow, vrow, t1, op.subtract)
                # srow = var + eps ; rrow ~= rsqrt(srow) ; newton polish
                nc.vector.tensor_scalar(srow, vrow, EPS, None, op.add)
                nc.scalar.activation(rrow, srow, AF.Sqrt)
                nc.vector.reciprocal(t1, rrow)
                # newton: r = r*(1.5 - 0.5*s*r*r)
                nc.vector.tensor_tensor(rrow, t1, t1, op.mult)
                nc.vector.tensor_tensor(rrow, rrow, srow, op.mult)
                nc.vector.tensor_scalar(rrow, rrow, -0.5, 1.5, op.mult, op.add)
                nc.vector.tensor_tensor(rrow, rrow, t1, op.mult)
                # broadcast mean, rstd to [128, T] (K=1 matmul), move to SBUF
                bm = psum(128, TW)
                br = psum(128, TW)
                nc.tensor.matmul(bm[:], (ones[0:1, :]), (mrow), start=True, stop=True)
                nc.tensor.matmul(br[:], (ones[0:1, :]), (rrow), start=True, stop=True)
                nc.scalar.activation(mrb[0][:], bm[:], AF.Copy)
                nc.scalar.activation(mrb[1][:], br[:], AF.Copy)
                for c in range(DC):
                    tt = trp.tile([128, TW], dt, tag="trn", name="trn")
                    nc.gpsimd.tensor_tensor(tt[:], src[c][:], mrb[0][:], op.subtract)
                    nc.gpsimd.tensor_tensor(tt[:], tt[:], mrb[1][:], op.mult)
                    nc.vector.tensor_scalar(dst[c][:], tt[:], g_ap(c), b_ap(c),
                                            op.mult, op.add)

            for l in range(L):
                nc.sync.dma_start(lnb[:], ln_d[l, :, :])
                nc.sync.dma_start(bsb[:], bs_d[l, :, :])

                layernorm(h, xn,
                          lambda c: lnb[:, c:c + 1],
                          lambda c: lnb[:, 6 + c:6 + c + 1])

                # ---- attention ----
                awt = [awp.tile([128, 3 * D], dt, tag="aw", name="aw") for c in range(DC)]
                for c in range(DC):
                    nc.sync.dma_start(awt[c][:], aw_d[l, c * 128:(c + 1) * 128, :])

                # q,k projections: 12 dout tiles [128, T]
                qk = [qkp.tile([128, TW], dt, tag="qk", name="qk") for _ in range(12)]
                for dtile in range(12):
                    ps = psum()
                    for c in range(DC):
                        nc.tensor.matmul(ps[:], (awt[c][:, dtile * 128:(dtile + 1) * 128]),
                                         (xn[c][:]), start=(c == 0), stop=(c == DC - 1))
                    nc.scalar.activation(qk[dtile][:], ps[:], AF.Identity,
                                         bias=bsb[:, dtile:dtile + 1])
                # vT: per head, two token tiles [128,64],[16,64]
                vt = []
                for hh in range(H):
                    row = []
                    for q in range(2):
                        tw = 128 if q == 0 else TW - 128
                        ps = psum(128, 64)
                        for c in range(DC):
                            nc.tensor.matmul(
                                ps[0:tw, :],
                                (xn[c][:, q * 128:q * 128 + tw]),
                                (awt[c][:, 2 * D + hh * HD:2 * D + (hh + 1) * HD]),
                                start=(c == 0), stop=(c == DC - 1))
                        sb = vtp.tile([128, 64], dt, tag="vt", name="vt")
                        nc.scalar.activation(sb[0:tw, :], ps[0:tw, :], AF.Copy)
                        row.append(sb)
                    vt.append(row)

                # scores + softmax + AV per head
                aout = [aop.tile([128, TW], dt, tag=f"ao{i}", name=f"ao{i}") for i in range(DC)]
                for hh in range(H):
                    dtile, ro = hh // 2, (hh % 2) * 64
                    q_ap = qk[dtile][ro:ro + 64, :]
                    k_ap = qk[6 + dtile][ro:ro + 64, :]
                    pT = [trp.tile([128, TW], dt, tag="trn", name="trn") for _ in range(2)]
                    for q in range(2):
                        tw = 128 if q == 0 else TW - 128
                        ps = psum()
                        nc.tensor.matmul(ps[0:tw, :], (q_ap[:, q * 128:q * 128 + tw]),
                                         (k_ap), start=True, stop=True)
                        ss = trp.tile([128, TW], dt, tag="trn", name="trn")
                        nc.vector.tensor_tensor(ss[0:tw, :], ps[0:tw, :],
                                                cmask[q][0:tw, :], op.add)
                        mx = trp.tile([128, 2], dt, tag="mx", name="mx")
                        nc.vector.reduce_max(mx[0:tw, 0:1], ss[0:tw, :], AX)
                        nc.vector.tensor_scalar(mx[0:tw, 1:2], mx[0:tw, 0:1],
                                                -0.125, None, op.mult)
                        pe_ = trp.tile([128, TW], dt, tag="trn", name="trn")
                        nc.scalar.activation(pe_[0:tw, :], ss[0:tw, :], AF.Exp,
                                             bias=mx[0:tw, 1:2], scale=0.125)
                        nc.vector.reduce_sum(mx[0:tw, 0:1], pe_[0:tw, :], AX)
                        nc.vector.reciprocal(mx[0:tw, 0:1], mx[0:tw, 0:1])
                        nc.vector.tensor_scalar(pe_[0:tw, :], pe_[0:tw, :],
                                                mx[0:tw, 0:1], None, op.mult)
                        # transpose pe_[0:tw, 0:T] into pT tiles
                        for kb in range(2):
                            kw = 128 if kb == 0 else TW - 128
                            pst = psum(128, 128)
                            nc.tensor.transpose(pst[0:kw, 0:tw],
                                                pe_[0:tw, kb * 128:kb * 128 + kw],
                                                iden[0:tw, 0:tw])
                            nc.vector.tensor_copy(
                                pT[kb][0:kw, q * 128:q * 128 + tw],
                                pst[0:kw, 0:tw])
                    pa = psum(128, TW)
                    for kb in range(2):
                        kw = 128 if kb == 0 else TW - 128
                        nc.tensor.matmul(pa[0:64, :], (vt[hh][kb][0:kw, :]),
                                         (pT[kb][0:kw, :]), start=(kb == 0),
                                         stop=(kb == 1))
                    nc.scalar.activation(aout[dtile][ro:ro + 64, :], pa[0:64, :],
                                         AF.Copy)

                # proj + residual
                pwt = [w768p.tile([128, D], dt, tag="w768", name="w768") for _ in range(DC)]
                for c in range(DC):
                    nc.sync.dma_start(pwt[c][:], pw_d[l, c * 128:(c + 1) * 128, :])
                for co in range(DC):
                    ps = psum()
                    for c in range(DC):
                        nc.tensor.matmul(ps[:], (pwt[c][:, co * 128:(co + 1) * 128]),
                                         (aout[c][:]), start=(c == 0), stop=(c == DC - 1))
                    nc.vector.scalar_tensor_tensor(h[co][:], ps[:],
                                                   bsb[:, 12 + co:12 + co + 1],
                                                   h[co][:], op.add, op.add)

                layernorm(h, xn,
                          lambda c: lnb[:, 12 + c:12 + c + 1],
                          lambda c: lnb[:, 18 + c:18 + c + 1])

                # fc + gelu
                ge = [gep.tile([128, TW], dt, tag="ge", name="ge") for _ in range(FT)]
                for half in range(2):
                    fwt = [fwp.tile([128, FF // 2], dt, tag="fw", name="fw") for _ in range(DC)]
                    for c in range(DC):
                        nc.sync.dma_start(
                            fwt[c][:],
                            fw_d[l, c * 128:(c + 1) * 128,
                                 half * (FF // 2):(half + 1) * (FF // 2)])
                    for ft in range(half * 12, half * 12 + 12):
                        fo = ft * 128 - half * (FF // 2)
                        ps = psum()
                        for c in range(DC):
                            nc.tensor.matmul(ps[:], (fwt[c][:, fo:fo + 128]),
                                             (xn[c][:]), start=(c == 0),
                                             stop=(c == DC - 1))
                        nc.scalar.activation(ge[ft][:], ps[:], AF.Gelu_apprx_tanh,
                                             bias=bsb[:, 18 + ft:18 + ft + 1])
                # fcp + residual, staged in 4 groups of 6 k-chunks
                psf = [psacc() for co in range(DC)]
                for gg in range(4):
                    gwt = [w768p.tile([128, D], dt, tag="w768", name="w768") for _ in range(6)]
                    for i in range(6):
                        kk = gg * 6 + i
                        nc.sync.dma_start(gwt[i][:],
                                          gw_d[l, kk * 128:(kk + 1) * 128, :])
                    for co in range(DC):
                        for i in range(6):
                            kk = gg * 6 + i
                            nc.tensor.matmul(
                                psf[co][:], (gwt[i][:, co * 128:(co + 1) * 128]),
                                (ge[kk][:]), start=(gg == 0 and i == 0),
                                stop=(gg == 3 and i == 5))
                for co in range(DC):
                    nc.vector.scalar_tensor_tensor(h[co][:], psf[co][:],
                                                   bsb[:, 42 + co:42 + co + 1],
                                                   h[co][:], op.add, op.add)

            # final LN + LM head
            nc.sync.dma_start(lnb[:, 0:12], lnf_d[:, :])
            layernorm(h, xn,
                      lambda c: lnb[:, c:c + 1],
                      lambda c: lnb[:, 6 + c:6 + c + 1])
            for vtile in range(17):
                wvt = w768p.tile([128, D], dt, tag="w768", name="w768")
                # wteT slice [768, 128] -> need [128(dchunk) x ...]: DMA per chunk
                lg = gep.tile([128, TW], dt, tag="ge", name="ge")
                ps = psum(128, 17)
                for c in range(DC):
                    nc.sync.dma_start(
                        wvt[:, c * 128:(c + 1) * 128],
                        wt_d[c * 128:(c + 1) * 128, vtile * 128:(vtile + 1) * 128])
                for c in range(DC):
                    nc.tensor.matmul(ps[:], (wvt[:, c * 128:(c + 1) * 128]),
                                     (xn[c][:, P - 1:P + 16]),
                                     start=(c == 0), stop=(c == DC - 1))
                nc.scalar.activation(lg[:, 0:17], ps[:], AF.Copy)
                nc.sync.dma_start(out_d[vtile, :, :], lg[:, 0:17])

    nc.compile()
    return nc


def _prep_static(wte, wpe, ln1_g, ln1_b, attn_w, proj_w, fc_w, fcp_w,
                 ln2_g, ln2_b, lnf_g, lnf_b, attn_b, proj_b, fc_b, fcp_b):
    f32 = np.float32
    ln_all = np.zeros((L, 128, 24), f32)
    bias_all = np.zeros((L, 128, 48), f32)
    for l in range(L):
        ln_all[l, :, 0:6] = ln1_g[l].reshape(6, 128).T
        ln_all[l, :, 6:12] = ln1_b[l].reshape(6, 128).T
        ln_all[l, :, 12:18] = ln2_g[l].reshape(6, 128).T
        ln_all[l, :, 18:24] = ln2_b[l].reshape(6, 128).T
        bias_all[l, :, 0:12] = attn_b[l, :2 * D].reshape(12, 128).T
        bias_all[l, :, 12:18] = proj_b[l].reshape(6, 128).T
        bias_all[l, :, 18:42] = fc_b[l].reshape(24, 128).T
        bias_all[l, :, 42:48] = fcp_b[l].reshape(6, 128).T
    lnf_all = np.zeros((128, 12), f32)
    lnf_all[:, 0:6] = lnf_g.reshape(6, 128).T
    lnf_all[:, 6:12] = lnf_b.reshape(6, 128).T
    wteT = np.zeros((D, VPAD), f32)
    wteT[:, :V] = wte.T
    cmask = np.zeros((2, 128, TW), f32)
    for q in range(2):
        for p_ in range(128):
            qpos = q * 128 + p_
            cmask[q, p_, qpos + 1:] = NEG
    ones = np.ones((128, 128), f32)
    iden = np.eye(128, dtype=f32)
    return dict(attn_w=np.ascontiguousarray(attn_w, f32),
                proj_w=np.ascontiguousarray(proj_w, f32),
                fc_w=np.ascontiguousarray(fc_w, f32),
                fcp_w=np.ascontiguousarray(fcp_w, f32),
                wteT=wteT, ln_all=ln_all, lnf_all=lnf_all,
                bias_all=bias_all, ones=ones, iden=iden, cmask=cmask)


def kernel(wte, wpe, ln1_g, ln1_b, attn_w, attn_b, proj_w, proj_b,
           ln2_g, ln2_b, fc_w, fc_b, fcp_w, fcp_b, lnf_g, lnf_b,
           first_idx, pos_ids, max_len, topk, _collect=None):
    import jax
    import jax.numpy as jnp
    from concourse.bass_utils import run_bass_kernel_spmd

    wte = np.asarray(wte, np.float32)
    wpe = np.asarray(wpe, np.float32)
    first_idx = np.asarray(first_idx, np.int32)
    max_len = int(max_len)
    kk = min(int(topk), V)
    assert max_len == S and first_idx.shape == (1, P)

    if "nc" not in _CACHE:
        _CACHE["nc"] = _build_nc()
    nc = _CACHE["nc"]

    static = _prep_static(np.asarray(wte), wpe,
                          np.asarray(ln1_g, np.float32), np.asarray(ln1_b, np.float32),
                          np.asarray(attn_w, np.float32), np.asarray(proj_w, np.float32),
                          np.asarray(fc_w, np.float32), np.asarray(fcp_w, np.float32),
                          np.asarray(ln2_g, np.float32), np.asarray(ln2_b, np.float32),
                          np.asarray(lnf_g, np.float32), np.asarray(lnf_b, np.float32),
                          np.asarray(attn_b, np.float32), np.asarray(proj_b, np.float32),
                          np.asarray(fc_b, np.float32), np.asarray(fcp_b, np.float32))

    pos_full = np.arange(T, dtype=np.int32)
    buf = np.zeros(T, np.int32)
    buf[:P] = first_idx[0]
    cpu0 = jax.devices("cpu")[0]
    with jax.default_device(cpu0):
        keys = jax.random.split(jax.random.key(1), S)
    band = np.arange(V)
    inf_mask = (band <= NUM_SEM) | (band == NUM_SEM + 1 + NUM_AC)

    cpu = jax.devices("cpu")[0]
    step_logits = np.zeros((S, 1, V), np.float32)

    for s in range(S):
        x = wte[buf] + wpe[pos_full]          # [T, 768]
        xp = np.zeros((TW, D), np.float32)
        xp[:T] = x
        x6 = np.ascontiguousarray(xp.T.reshape(DC, 128, TW), np.float32)
        in_map = dict(static, x6=x6)
        import os as _os
        _tr = bool(_os.environ.get("BASS_KERNEL_TRACE")) and s == 0
        res = run_bass_kernel_spmd(nc, [in_map], core_ids=[0], trace=_tr)
        lg = res.results[0]["logits"]          # [17, 128, T]
        if _collect is not None:
            _collect.append(res)
        cur = P + s
        row = lg[:, :, s].reshape(VPAD)[:V].astype(np.float32)
        row = row * TEMP
        row = np.where(inf_mask, -np.inf, row)
        thresh = np.partition(row, V - kk)[V - kk]
        row = np.where(row < thresh, -np.inf, row).astype(np.float32)
        step_logits[s, 0] = row
        with jax.default_device(cpu):
            samp = int(jax.random.categorical(keys[s], jnp.asarray(row)[None, :])[0])
        buf[cur] = samp

    idx_buf = buf[None, :].astype(np.int32)
    return idx_buf, step_logits
